# revision 28
# baseline (speedup 1.0000x reference)
"""Fused pre-LN multi-head self-attention block for Trainium2, SPMD over 8 NeuronCores.

Strategy (Megatron-style head parallelism):
  - Each core owns 2 of the 16 heads (a 128-wide slice of the QKV output dims)
    for BOTH batches, and computes a partial dense-projection output; the host
    sums the 8 partials and adds bd.
  - hidden_states is shipped transposed (xT [1024, 4096], bf16) so LayerNorm
    statistics are computed with ones-matmuls on the PE and the normalization
    itself is algebraically folded into the QKV projections:
        q = rstd * (x @ WqeT - mu * wqsum) + bqe
    with Wqe = Wq_slice * gamma * scale (host-folded), bqe = Wq_slice @ beta + bq.
  - Scores are computed transposed (sT[k,q] = kT.T @ qT); softmax skips the
    max-subtraction (inputs are standard-normal scale); the denominator comes
    from a ones-column inside the padded V tiles in the probs@V matmul.
  - ScalarE runs exactly one activation function per phase (Sqrt in phase 1,
    Exp in phase 2) so each phase loads its ACT table set once; both
    reciprocals (1/std, 1/denominator) use the single-op DVE
    reciprocal_approx_fast (~18-bit accurate, ~5x faster than the iterative
    DVE reciprocal that dominated the old kernel's vector-engine time).
  - x^2 for the variance stats is split between VectorE and GpSimd to
    balance engine load in phase 1.
  - PSUM pools are scoped per phase: phase 1 (stats 2 + qkv 3 + vT 2 banks)
    is released before phase 2 allocates (scores 4 + ctx 2 + dense 2 banks),
    so each phase gets the banks it needs without static over-commit.
  - V tiles are zero-padded to 128 lhsT columns (ones col at 64) so the
    probs@V weight loads qualify for fast-weight-load and overlap matmuls.
"""

import sys

sys.path.insert(0, "/opt/trn_rl_repo")

import numpy as np
import ml_dtypes

B, S, HID = 2, 2048, 1024
NH, HD = 16, 64
EPS = 1e-12
NCORES = 8
DL = HID // NCORES          # 128 local q/k/v dims (2 heads) per core
PB = B * S                  # 4096 total positions
SCALE = 1.0 / np.sqrt(HD)   # combined q*k score scale (1/8), folded into Wq
NPOSG = PB // 512           # 8 position groups of 512
KC = S // 128               # 16 key chunks per batch
QG = S // 512               # 4 query groups per batch

_BUILT = {}
last_launch = {}


def _build(with_bias, fused_mask):
    import concourse.tile as tile
    from concourse import bacc, mybir
    from contextlib import ExitStack

    F32 = mybir.dt.float32
    BF16 = mybir.dt.bfloat16
    AF = mybir.ActivationFunctionType
    OP = mybir.AluOpType

    nc = bacc.Bacc("TRN2", target_bir_lowering=False, debug=False)

    xT = nc.dram_tensor("xT", [HID, PB], BF16, kind="ExternalInput")
    wq = nc.dram_tensor("wq", [HID, DL], BF16, kind="ExternalInput")
    wk = nc.dram_tensor("wk", [HID, DL], BF16, kind="ExternalInput")
    wv = nc.dram_tensor("wv", [HID, DL], BF16, kind="ExternalInput")
    wd = nc.dram_tensor("wd", [DL, HID], BF16, kind="ExternalInput")
    # raw bf16-weight column sums as three [1, DL] rows (q, k, v) for the
    # rank-1 mean-correction matmul
    wsumsT = nc.dram_tensor("wsumsT", [1, 3 * DL], BF16, kind="ExternalInput")
    if not fused_mask:
        madd = nc.dram_tensor("madd", [128, B * KC], F32, kind="ExternalInput")
    if with_bias:
        bqkv = nc.dram_tensor("bqkv", [DL, 3], F32, kind="ExternalInput")
    F16 = mybir.dt.float16
    out = nc.dram_tensor("out", [PB, HID], F16, kind="ExternalOutput")

    with tile.TileContext(nc) as tc, ExitStack() as ctx:
        consts = ctx.enter_context(tc.tile_pool(name="consts", bufs=1))
        persist = ctx.enter_context(tc.tile_pool(name="persist", bufs=1))
        xpool = ctx.enter_context(tc.tile_pool(name="xpool", bufs=2))
        sqp = ctx.enter_context(tc.tile_pool(name="sqp", bufs=2))
        rowp = ctx.enter_context(tc.tile_pool(name="rowp", bufs=8))
        bcp = ctx.enter_context(tc.tile_pool(name="bcp", bufs=2))
        epp = ctx.enter_context(tc.tile_pool(name="epp", bufs=3))
        etp = ctx.enter_context(tc.tile_pool(name="etp", bufs=4))
        dnp = ctx.enter_context(tc.tile_pool(name="dnp", bufs=2))
        obp = ctx.enter_context(tc.tile_pool(name="obp", bufs=3))

        # ---- constants / weights
        ones_col = consts.tile([128, 1], BF16)
        nc.vector.memset(ones_col[:], 1.0)
        ident = consts.tile([128, 128], BF16)
        from concourse.masks import make_identity
        make_identity(nc, ident[:])
        eps_t = consts.tile([1, 1], F32)
        nc.vector.memset(eps_t[:], EPS)
        wsumsT_sb = consts.tile([1, 3 * DL], BF16)
        nc.sync.dma_start(out=wsumsT_sb[:], in_=wsumsT[:, :])
        if not fused_mask:
            madd_sb = consts.tile([128, B * KC], F32)
            nc.sync.dma_start(out=madd_sb[:], in_=madd[:, :])
        if with_bias:
            bqkv_sb = consts.tile([DL, 3], F32)
            nc.sync.dma_start(out=bqkv_sb[:], in_=bqkv[:, :])

        xT_r = xT.rearrange("(hc p) q -> p hc q", p=128)
        x_first = xpool.tile([128, 8, 512], BF16, tag="x")
        nc.sync.dma_start(out=x_first[:], in_=xT_r[:, :, 0:512])

        wq_sb = persist.tile([128, 8, DL], BF16)
        wk_sb = persist.tile([128, 8, DL], BF16)
        wv_sb = persist.tile([128, 8, DL], BF16)
        for wsb, wdr in ((wq_sb, wq), (wk_sb, wk), (wv_sb, wv)):
            nc.sync.dma_start(
                out=wsb[:], in_=wdr.rearrange("(hc p) d -> p hc d", p=128)
            )
        wd_sb = persist.tile([DL, HID], BF16)
        nc.sync.dma_start(out=wd_sb[:], in_=wd[:, :])

        qT_sb = persist.tile([128, PB], BF16)
        kT_sb = persist.tile([128, PB], BF16)
        # v tiles padded to 128 lhsT columns: [0:64]=v, [64]=ones, [65:128]=0
        vA_sb = persist.tile([128, B * KC, 128], BF16)
        vB_sb = persist.tile([128, B * KC, 128], BF16)
        for vsb in (vA_sb, vB_sb):
            nc.vector.memset(vsb[:, :, HD : HD + 1], 1.0)
            nc.vector.memset(vsb[:, :, HD + 1 : 128], 0.0)
        ctxT_sb = persist.tile([128, PB], BF16)

        # ctx-accumulator pool outlives both phases: its banks host warmup
        # matmuls that keep the PE (and its HAM clock) busy across the
        # phase-1 -> phase-2 pool-release barrier.
        ctps = ctx.enter_context(tc.tile_pool(name="ctps", bufs=1, space="PSUM"))

        # ================= phase 1: LN stats + QKV projections ===============
        # software-pipelined: stats(pg) runs on the PE before qkv(pg-1), so
        # the rank-1 mean-correction matmul of pg-1 never waits on the DVE
        # row chain (in-order PE never stalls).
        with tc.tile_pool(name="ph1ps", bufs=1, space="PSUM") as ph1:
            x_prev = x_first
            carry = None  # (xa, rstd_b, mu_neg, ps) of the previous pg

            def emit_qkv(xa, rstd_b, mu_neg, ps):
                vT_blk = epp.tile([128, 512], BF16, tag="vT")
                for (w_sb, wcol, target) in (
                    (wq_sb, 0, qT_sb[:, ps : ps + 512]),
                    (wk_sb, 1, kT_sb[:, ps : ps + 512]),
                    (wv_sb, 2, vT_blk[:]),
                ):
                    mm_ps = ph1.tile([128, 512], F32, tag="mm", bufs=2)
                    for hc in range(8):
                        nc.tensor.matmul(
                            mm_ps[:], lhsT=w_sb[:, hc, :], rhs=xa[:, hc, :],
                            start=(hc == 0), stop=False,
                        )
                    nc.tensor.matmul(
                        mm_ps[:],
                        lhsT=wsumsT_sb[:, wcol * DL : (wcol + 1) * DL],
                        rhs=mu_neg[:], start=False, stop=True,
                    )
                    # target = raw_c * rstd
                    if with_bias:
                        t2 = epp.tile([128, 512], F32, tag="ep2")
                        nc.vector.scalar_tensor_tensor(
                            out=t2[:], in0=rstd_b[:], scalar=1.0,
                            in1=mm_ps[:], op0=OP.mult, op1=OP.mult,
                        )
                        nc.vector.tensor_scalar_add(
                            out=target, in0=t2[:], scalar1=bqkv_sb[:, wcol : wcol + 1]
                        )
                    else:
                        nc.vector.scalar_tensor_tensor(
                            out=target, in0=rstd_b[:], scalar=1.0,
                            in1=mm_ps[:], op0=OP.mult, op1=OP.mult,
                        )
                # transpose vT -> v (per 128-pos chunk) into padded v tiles
                for c4 in range(4):
                    j = (ps // 512) * 4 + c4  # global 128-chunk == b*KC + kc
                    tp_ps = ph1.tile([128, 128], BF16, tag="vt", bufs=2)
                    nc.tensor.transpose(
                        tp_ps[:, :], vT_blk[:, c4 * 128 : (c4 + 1) * 128], ident[:]
                    )
                    nc.vector.tensor_copy(vA_sb[:, j, 0:HD], tp_ps[:, 0:HD])
                    nc.vector.tensor_copy(vB_sb[:, j, 0:HD], tp_ps[:, HD : 2 * HD])

            for pg in range(NPOSG):
                ps = pg * 512
                xa = x_prev
                if pg + 1 < NPOSG:
                    x_nxt = xpool.tile([128, 8, 512], BF16, tag="x")
                    nc.sync.dma_start(
                        out=x_nxt[:], in_=xT_r[:, :, ps + 512 : ps + 1024]
                    )
                    x_prev = x_nxt

                # --- stats chain (sum in psum row 0, sumsq in row 32);
                # x^2 in two wide DVE muls (cheaper than 8 narrow ones)
                stats_ps = ph1.tile([64, 512], F32, tag="stats", bufs=2)
                xsq_h = [
                    sqp.tile([128, 4, 512], BF16, tag="xsq", name=f"xsq{h}")
                    for h in range(2)
                ]
                for h in range(2):
                    nc.vector.tensor_mul(
                        xsq_h[h][:], xa[:, 4 * h : 4 * h + 4, :],
                        xa[:, 4 * h : 4 * h + 4, :],
                    )
                for hc in range(8):
                    st = hc == 0
                    sp = hc == 7
                    nc.tensor.matmul(
                        stats_ps[0:1, :], lhsT=ones_col[:], rhs=xa[:, hc, :],
                        start=st, stop=sp, skip_group_check=True,
                    )
                    nc.tensor.matmul(
                        stats_ps[32:33, :], lhsT=ones_col[:],
                        rhs=xsq_h[hc // 4][:, hc % 4, :],
                        start=st, stop=sp, skip_group_check=True,
                    )

                # row stats: s1 (sum); rstd = 1/sqrt(var+eps)
                s1_row = rowp.tile([1, 512], F32, tag="s1")
                nc.vector.tensor_copy(s1_row[:], stats_ps[0:1, :])
                s1sq_row = rowp.tile([1, 512], F32, tag="s1sq")
                nc.vector.tensor_mul(s1sq_row[:], s1_row[:], s1_row[:])
                u_row = rowp.tile([1, 512], F32, tag="u")
                nc.vector.scalar_tensor_tensor(
                    out=u_row[:], in0=s1sq_row[:], scalar=1.0 / HID,
                    in1=stats_ps[32:33, :], op0=OP.mult, op1=OP.subtract,
                )  # u = s1^2/HID - s2  (= -HID*var)
                std_row = rowp.tile([1, 512], F32, tag="std")
                nc.scalar.activation(
                    std_row[:], u_row[:], AF.Sqrt, bias=eps_t[:], scale=-1.0 / HID
                )  # sqrt(var + eps)
                rstd_row = rowp.tile([1, 512], F32, tag="rstd")
                nc.vector.reciprocal_approx_fast(out=rstd_row[:], in_=std_row[:])
                mu_neg = rowp.tile([1, 512], BF16, tag="mu_neg")
                nc.vector.tensor_scalar_mul(
                    out=mu_neg[:], in0=s1_row[:], scalar1=-1.0 / HID
                )
                rstd_b = bcp.tile([128, 512], F32, tag="rstd_b")
                nc.gpsimd.partition_broadcast(rstd_b[:], rstd_row[:])

                if carry is not None:
                    emit_qkv(*carry)
                carry = (xa, rstd_b, mu_neg, ps)
            emit_qkv(*carry)

        # ================= phase 2: attention + pipelined dense ==============
        # keep the PE warm across the pool-release barrier: junk matmuls into
        # a ctx slot (later overwritten by the first start=True accumulate)
        warm_ps = ctps.tile([128, 512], F32, tag="ctx", bufs=2)
        for w in range(12):
            nc.tensor.matmul(
                warm_ps[:], lhsT=ident[:], rhs=qT_sb[:, 512 * (w % 8) : 512 * (w % 8) + 512],
                start=True, stop=True,
            )

        with tc.tile_pool(name="scps", bufs=1, space="PSUM") as scps:

            def emit_dense(qs):
                for c4 in range(4):
                    pc = qs + c4 * 128
                    for half in range(2):
                        ops_ = scps.tile([128, 512], F32, tag="sc", bufs=3, name="dsq")
                        nc.tensor.matmul(
                            ops_[:], lhsT=ctxT_sb[:, pc : pc + 128],
                            rhs=wd_sb[:, half * 512 : (half + 1) * 512],
                            start=True, stop=True,
                        )
                        osb = obp.tile([128, 512], F16, tag="ob")
                        nc.vector.tensor_copy(osb[:], ops_[:])
                        nc.sync.dma_start(
                            out=out[pc : pc + 128, half * 512 : (half + 1) * 512],
                            in_=osb[:],
                        )

            for b in range(B):
                for qg in range(QG):
                    qs = b * S + qg * 512
                    ctxA_ps = ctps.tile([128, 512], F32, tag="ctx", bufs=2)
                    ctxB_ps = ctps.tile([128, 512], F32, tag="ctx", bufs=2)
                    # A/B score matmuls adjacent: disjoint row groups let
                    # the PE pull the next LDWEIGHTS ahead and run the pair
                    # concurrently; 3 sc slots keep adjacency through slot
                    # recycling.
                    for kc2 in range(KC // 2):
                        kc = 2 * kc2
                        ks = b * S + kc * 128
                        st = kc == 0
                        sp2 = kc + 1 == KC - 1
                        psA = scps.tile([128, 1024], F32, tag="sc", bufs=3)
                        psB = scps.tile([128, 1024], F32, tag="sc", bufs=3)
                        for j in range(2):
                            nc.tensor.matmul(
                                psA[:, 512 * j : 512 * (j + 1)],
                                lhsT=kT_sb[0:64, ks + 128 * j : ks + 128 * (j + 1)],
                                rhs=qT_sb[0:64, qs : qs + 512],
                                start=True, stop=True,
                            )
                            nc.tensor.matmul(
                                psB[:, 512 * j : 512 * (j + 1)],
                                lhsT=kT_sb[64:128, ks + 128 * j : ks + 128 * (j + 1)],
                                rhs=qT_sb[64:128, qs : qs + 512],
                                start=True, stop=True,
                            )
                        eA = etp.tile([128, 1024], BF16, tag="e")
                        eB = etp.tile([128, 1024], BF16, tag="e")
                        if fused_mask:
                            nc.scalar.activation(eA[:], psA[:], AF.Exp)
                            nc.scalar.activation(eB[:], psB[:], AF.Exp)
                        else:
                            for j in range(2):
                                mcol = madd_sb[:, b * KC + kc + j : b * KC + kc + j + 1]
                                nc.scalar.activation(
                                    eA[:, 512 * j : 512 * (j + 1)],
                                    psA[:, 512 * j : 512 * (j + 1)],
                                    AF.Exp, bias=mcol, scale=1.0,
                                )
                                nc.scalar.activation(
                                    eB[:, 512 * j : 512 * (j + 1)],
                                    psB[:, 512 * j : 512 * (j + 1)],
                                    AF.Exp, bias=mcol, scale=1.0,
                                )
                        for j in range(2):
                            nc.tensor.matmul(
                                ctxA_ps[:, :],
                                lhsT=vA_sb[:, b * KC + kc + j, :],
                                rhs=eA[:, 512 * j : 512 * (j + 1)],
                                start=(st and j == 0), stop=(sp2 and j == 1),
                            )
                        for j in range(2):
                            nc.tensor.matmul(
                                ctxB_ps[:, :],
                                lhsT=vB_sb[:, b * KC + kc + j, :],
                                rhs=eB[:, 512 * j : 512 * (j + 1)],
                                start=(st and j == 0), stop=(sp2 and j == 1),
                            )

                    # evacuate ctx PSUM fast (frees the accum slots for the
                    # next query group), then normalize from the SBUF copies:
                    # rdn = exp(-ln(dn)) with dn in row HD of each copy.
                    # head A ctx -> partitions 0:64, head B ctx -> 64:128 so the
                    # normalize muls have partition-aligned SBUF operands
                    cAB = dnp.tile([128, 512], F32, tag="cs", bufs=2)
                    nc.vector.tensor_copy(cAB[0:HD, :], ctxA_ps[0:HD, :])
                    nc.vector.tensor_copy(cAB[HD : 2 * HD, :], ctxB_ps[0:HD, :])
                    dn_row = dnp.tile([1, 1024], F32, tag="dn_row", bufs=2)
                    nc.vector.tensor_copy(dn_row[:, 0:512], ctxA_ps[HD : HD + 1, :])
                    nc.vector.tensor_copy(dn_row[:, 512:1024], ctxB_ps[HD : HD + 1, :])
                    rdn_row = dnp.tile([1, 1024], F32, tag="rdn_row", bufs=1)
                    nc.vector.reciprocal_approx_fast(out=rdn_row[:], in_=dn_row[:])
                    rdn_b = dnp.tile([128, 1024], F32, tag="rdn_b", bufs=1)
                    nc.gpsimd.partition_broadcast(rdn_b[:], rdn_row[:])
                    nc.vector.tensor_mul(
                        ctxT_sb[0:HD, qs : qs + 512],
                        cAB[0:HD, :], rdn_b[0:HD, 0:512],
                    )
                    nc.vector.tensor_mul(
                        ctxT_sb[HD : 2 * HD, qs : qs + 512],
                        cAB[HD : 2 * HD, :], rdn_b[HD : 2 * HD, 512:1024],
                    )
                    emit_dense(qs)
    nc.compile()
    return nc


def _get_nc(with_bias, fused_mask):
    key = (bool(with_bias), bool(fused_mask))
    if key not in _BUILT:
        _BUILT[key] = _build(*key)
    return _BUILT[key]


def kernel(
    hidden_states,
    attention_mask,
    Wq, bq, Wk, bk, Wv, bv, Wd, bd,
    ln_gamma, ln_beta,
):
    from concourse.bass_utils import run_bass_kernel_spmd

    hidden_states = np.asarray(hidden_states, dtype=np.float32)
    attention_mask = np.asarray(attention_mask, dtype=np.float32)
    Wq, bq = np.asarray(Wq, np.float32), np.asarray(bq, np.float32)
    Wk, bk = np.asarray(Wk, np.float32), np.asarray(bk, np.float32)
    Wv, bv = np.asarray(Wv, np.float32), np.asarray(bv, np.float32)
    Wd, bd = np.asarray(Wd, np.float32), np.asarray(bd, np.float32)
    gamma = np.asarray(ln_gamma, np.float32)
    beta = np.asarray(ln_beta, np.float32)

    x2d = hidden_states.reshape(PB, HID)
    xT = np.ascontiguousarray(x2d.T).astype(ml_dtypes.bfloat16)

    ma = (-1000.0 * (1.0 - attention_mask)).astype(np.float32)  # [B, S]
    madd = np.ascontiguousarray(
        ma.reshape(B, KC, 128).transpose(2, 0, 1).reshape(128, B * KC)
    )
    fused_mask = not np.any(ma != 0)

    in_maps = []
    biases_eff = []
    for p in range(NCORES):
        sl = slice(DL * p, DL * (p + 1))
        wq_e = Wq[sl, :] * gamma[None, :] * np.float32(SCALE)
        wk_e = Wk[sl, :] * gamma[None, :]
        wv_e = Wv[sl, :] * gamma[None, :]
        wq_b = np.ascontiguousarray(wq_e.T).astype(ml_dtypes.bfloat16)
        wk_b = np.ascontiguousarray(wk_e.T).astype(ml_dtypes.bfloat16)
        wv_b = np.ascontiguousarray(wv_e.T).astype(ml_dtypes.bfloat16)
        # raw column sums of the bf16 weights actually used on device,
        # as three [1, DL] rows for the rank-1 mean-correction matmul
        wsumsT = np.concatenate(
            [
                wq_b.astype(np.float32).sum(axis=0),
                wk_b.astype(np.float32).sum(axis=0),
                wv_b.astype(np.float32).sum(axis=0),
            ]
        ).reshape(1, 3 * DL).astype(ml_dtypes.bfloat16)
        b_eff = np.stack(
            [
                (Wq[sl, :] @ beta + bq[sl]) * np.float32(SCALE),
                Wk[sl, :] @ beta + bk[sl],
                Wv[sl, :] @ beta + bv[sl],
            ],
            axis=1,
        ).astype(np.float32)
        biases_eff.append(b_eff)
        wd_s = np.ascontiguousarray(Wd[:, sl].T).astype(ml_dtypes.bfloat16)
        in_maps.append(
            {
                "xT": xT,
                "wq": wq_b,
                "wk": wk_b,
                "wv": wv_b,
                "wd": wd_s,
                "wsumsT": wsumsT,
            }
        )

    with_bias = any(np.any(b != 0) for b in biases_eff)
    if with_bias:
        for p in range(NCORES):
            in_maps[p]["bqkv"] = biases_eff[p]
    if not fused_mask:
        for p in range(NCORES):
            in_maps[p]["madd"] = madd

    nc = _get_nc(with_bias, fused_mask)
    last_launch["nc"] = nc
    last_launch["in_maps"] = in_maps
    res = run_bass_kernel_spmd(nc, in_maps, core_ids=list(range(NCORES)))
    acc = res.results[0]["out"].astype(np.float32).copy()
    for p in range(1, NCORES):
        acc += res.results[p]["out"]
    acc += bd[None, :]
    return acc.reshape(B, S, HID)


# revision 29
# speedup vs baseline: 1.3603x; 1.3603x over previous
"""Fused pre-LN multi-head self-attention block for Trainium2, SPMD over 8 NeuronCores.

Strategy (Megatron-style head parallelism):
  - Each core owns 2 of the 16 heads (a 128-wide slice of the QKV output dims)
    for BOTH batches, and computes a partial dense-projection output; the host
    sums the 8 partials and adds bd.
  - hidden_states is shipped transposed (xT [1024, 4096], bf16) so LayerNorm
    statistics are computed with ones-matmuls on the PE and the normalization
    itself is algebraically folded into the QKV projections:
        q = rstd * (x @ WqeT - mu * wqsum) + bqe
    with Wqe = Wq_slice * gamma * scale (host-folded), bqe = Wq_slice @ beta + bq.
  - Scores are computed transposed (sT[k,q] = kT.T @ qT); softmax skips the
    max-subtraction (inputs are standard-normal scale); the denominator comes
    from a ones-column inside the padded V tiles in the probs@V matmul.
  - ScalarE runs exactly one activation function per phase (Sqrt in phase 1,
    Exp in phase 2) so each phase loads its ACT table set once; both
    reciprocals (1/std, 1/denominator) use the single-op DVE
    reciprocal_approx_fast (~18-bit accurate, ~5x faster than the iterative
    DVE reciprocal that dominated the old kernel's vector-engine time).
  - x^2 for the variance stats is split between VectorE and GpSimd to
    balance engine load in phase 1.
  - PSUM pools are scoped per phase: phase 1 (stats 2 + qkv 3 + vT 2 banks)
    is released before phase 2 allocates (scores 4 + ctx 2 + dense 2 banks),
    so each phase gets the banks it needs without static over-commit.
  - V tiles are zero-padded to 128 lhsT columns (ones col at 64) so the
    probs@V weight loads qualify for fast-weight-load and overlap matmuls.
"""

import sys

sys.path.insert(0, "/opt/trn_rl_repo")

import numpy as np
import ml_dtypes

B, S, HID = 2, 2048, 1024
NH, HD = 16, 64
EPS = 1e-12
NCORES = 8
DL = HID // NCORES          # 128 local q/k/v dims (2 heads) per core
PB = B * S                  # 4096 total positions
SCALE = 1.0 / np.sqrt(HD)   # combined q*k score scale (1/8), folded into Wq
NPOSG = PB // 512           # 8 position groups of 512
KC = S // 128               # 16 key chunks per batch
QG = S // 512               # 4 query groups per batch

_BUILT = {}
last_launch = {}


def _build(with_bias, fused_mask):
    import concourse.tile as tile
    from concourse import bacc, mybir
    from contextlib import ExitStack

    F32 = mybir.dt.float32
    BF16 = mybir.dt.bfloat16
    AF = mybir.ActivationFunctionType
    OP = mybir.AluOpType

    nc = bacc.Bacc("TRN2", target_bir_lowering=False, debug=False)

    xT = nc.dram_tensor("xT", [HID, PB], BF16, kind="ExternalInput")
    wq = nc.dram_tensor("wq", [HID, DL], BF16, kind="ExternalInput")
    wk = nc.dram_tensor("wk", [HID, DL], BF16, kind="ExternalInput")
    wv = nc.dram_tensor("wv", [HID, DL], BF16, kind="ExternalInput")
    wd = nc.dram_tensor("wd", [DL, HID], BF16, kind="ExternalInput")
    # raw bf16-weight column sums as three [1, DL] rows (q, k, v) for the
    # rank-1 mean-correction matmul
    wsumsT = nc.dram_tensor("wsumsT", [1, 3 * DL], BF16, kind="ExternalInput")
    if not fused_mask:
        madd = nc.dram_tensor("madd", [128, B * KC], F32, kind="ExternalInput")
    if with_bias:
        bqkv = nc.dram_tensor("bqkv", [DL, 3], F32, kind="ExternalInput")
    F16 = mybir.dt.float16
    out = nc.dram_tensor("out", [PB, HID], F16, kind="ExternalOutput")

    with tile.TileContext(nc) as tc, ExitStack() as ctx:
        consts = ctx.enter_context(tc.tile_pool(name="consts", bufs=1))
        persist = ctx.enter_context(tc.tile_pool(name="persist", bufs=1))
        xpool = ctx.enter_context(tc.tile_pool(name="xpool", bufs=2))
        sqp = ctx.enter_context(tc.tile_pool(name="sqp", bufs=2))
        rowp = ctx.enter_context(tc.tile_pool(name="rowp", bufs=8))
        bcp = ctx.enter_context(tc.tile_pool(name="bcp", bufs=2))
        epp = ctx.enter_context(tc.tile_pool(name="epp", bufs=3))
        etp = ctx.enter_context(tc.tile_pool(name="etp", bufs=4))
        dnp = ctx.enter_context(tc.tile_pool(name="dnp", bufs=2))
        obp = ctx.enter_context(tc.tile_pool(name="obp", bufs=3))

        # ---- constants / weights
        ones_col = consts.tile([128, 1], BF16)
        nc.vector.memset(ones_col[:], 1.0)
        ident = consts.tile([128, 128], BF16)
        from concourse.masks import make_identity
        make_identity(nc, ident[:])
        eps_t = consts.tile([1, 1], F32)
        nc.vector.memset(eps_t[:], EPS)
        wsumsT_sb = consts.tile([1, 3 * DL], BF16)
        nc.sync.dma_start(out=wsumsT_sb[:], in_=wsumsT[:, :])
        if not fused_mask:
            madd_sb = consts.tile([128, B * KC], F32)
            nc.sync.dma_start(out=madd_sb[:], in_=madd[:, :])
        if with_bias:
            bqkv_sb = consts.tile([DL, 3], F32)
            nc.sync.dma_start(out=bqkv_sb[:], in_=bqkv[:, :])

        xT_r = xT.rearrange("(hc p) q -> p hc q", p=128)
        x_first = xpool.tile([128, 8, 512], BF16, tag="x")
        nc.sync.dma_start(out=x_first[:], in_=xT_r[:, :, 0:512])

        wq_sb = persist.tile([128, 8, DL], BF16)
        wk_sb = persist.tile([128, 8, DL], BF16)
        wv_sb = persist.tile([128, 8, DL], BF16)
        for wsb, wdr in ((wq_sb, wq), (wk_sb, wk), (wv_sb, wv)):
            nc.sync.dma_start(
                out=wsb[:], in_=wdr.rearrange("(hc p) d -> p hc d", p=128)
            )
        wd_sb = persist.tile([DL, HID], BF16)
        nc.sync.dma_start(out=wd_sb[:], in_=wd[:, :])

        qT_sb = persist.tile([128, PB], BF16)
        kT_sb = persist.tile([128, PB], BF16)
        # v tiles padded to 128 lhsT columns: [0:64]=v, [64]=ones, [65:128]=0
        vA_sb = persist.tile([128, B * KC, 128], BF16)
        vB_sb = persist.tile([128, B * KC, 128], BF16)
        for vsb in (vA_sb, vB_sb):
            nc.vector.memset(vsb[:, :, HD : HD + 1], 1.0)
            nc.vector.memset(vsb[:, :, HD + 1 : 128], 0.0)
        ctxT_sb = persist.tile([128, PB], BF16)

        # ctx-accumulator pool outlives both phases: its banks host warmup
        # matmuls that keep the PE (and its HAM clock) busy across the
        # phase-1 -> phase-2 pool-release barrier.
        ctps = ctx.enter_context(tc.tile_pool(name="ctps", bufs=1, space="PSUM"))

        # ================= phase 1: LN stats + QKV projections ===============
        # software-pipelined: stats(pg) runs on the PE before qkv(pg-1), so
        # the rank-1 mean-correction matmul of pg-1 never waits on the DVE
        # row chain (in-order PE never stalls).
        with tc.tile_pool(name="ph1ps", bufs=1, space="PSUM") as ph1:
            x_prev = x_first
            carry = None  # (xa, rstd_b, mu_neg, ps) of the previous pg

            def emit_qkv(xa, rstd_b, mu_neg, ps):
                vT_blk = epp.tile([128, 512], BF16, tag="vT")
                for (w_sb, wcol, target) in (
                    (wq_sb, 0, qT_sb[:, ps : ps + 512]),
                    (wk_sb, 1, kT_sb[:, ps : ps + 512]),
                    (wv_sb, 2, vT_blk[:]),
                ):
                    mm_ps = ph1.tile([128, 512], F32, tag="mm", bufs=2)
                    for hc in range(8):
                        nc.tensor.matmul(
                            mm_ps[:], lhsT=w_sb[:, hc, :], rhs=xa[:, hc, :],
                            start=(hc == 0), stop=False,
                        )
                    nc.tensor.matmul(
                        mm_ps[:],
                        lhsT=wsumsT_sb[:, wcol * DL : (wcol + 1) * DL],
                        rhs=mu_neg[:], start=False, stop=True,
                    )
                    # target = raw_c * rstd
                    if with_bias:
                        t2 = epp.tile([128, 512], F32, tag="ep2")
                        nc.vector.scalar_tensor_tensor(
                            out=t2[:], in0=rstd_b[:], scalar=1.0,
                            in1=mm_ps[:], op0=OP.mult, op1=OP.mult,
                        )
                        nc.vector.tensor_scalar_add(
                            out=target, in0=t2[:], scalar1=bqkv_sb[:, wcol : wcol + 1]
                        )
                    else:
                        nc.vector.scalar_tensor_tensor(
                            out=target, in0=rstd_b[:], scalar=1.0,
                            in1=mm_ps[:], op0=OP.mult, op1=OP.mult,
                        )
                # transpose vT -> v (per 128-pos chunk) into padded v tiles
                for c4 in range(4):
                    j = (ps // 512) * 4 + c4  # global 128-chunk == b*KC + kc
                    tp_ps = ph1.tile([128, 128], BF16, tag="vt", bufs=2)
                    nc.tensor.transpose(
                        tp_ps[:, :], vT_blk[:, c4 * 128 : (c4 + 1) * 128], ident[:]
                    )
                    nc.vector.tensor_copy(vA_sb[:, j, 0:HD], tp_ps[:, 0:HD])
                    nc.vector.tensor_copy(vB_sb[:, j, 0:HD], tp_ps[:, HD : 2 * HD])

            for pg in range(NPOSG):
                ps = pg * 512
                xa = x_prev
                if pg + 1 < NPOSG:
                    x_nxt = xpool.tile([128, 8, 512], BF16, tag="x")
                    nc.sync.dma_start(
                        out=x_nxt[:], in_=xT_r[:, :, ps + 512 : ps + 1024]
                    )
                    x_prev = x_nxt

                # --- stats chain (sum in psum row 0, sumsq in row 32);
                # x^2 in two wide DVE muls (cheaper than 8 narrow ones)
                stats_ps = ph1.tile([64, 512], F32, tag="stats", bufs=2)
                xsq_h = [
                    sqp.tile([128, 4, 512], BF16, tag="xsq", name=f"xsq{h}")
                    for h in range(2)
                ]
                for h in range(2):
                    nc.vector.tensor_mul(
                        xsq_h[h][:], xa[:, 4 * h : 4 * h + 4, :],
                        xa[:, 4 * h : 4 * h + 4, :],
                    )
                for hc in range(8):
                    st = hc == 0
                    sp = hc == 7
                    nc.tensor.matmul(
                        stats_ps[0:1, :], lhsT=ones_col[:], rhs=xa[:, hc, :],
                        start=st, stop=sp, skip_group_check=True,
                    )
                    nc.tensor.matmul(
                        stats_ps[32:33, :], lhsT=ones_col[:],
                        rhs=xsq_h[hc // 4][:, hc % 4, :],
                        start=st, stop=sp, skip_group_check=True,
                    )

                # row stats: s1 (sum); rstd = 1/sqrt(var+eps)
                s1_row = rowp.tile([1, 512], F32, tag="s1")
                nc.vector.tensor_copy(s1_row[:], stats_ps[0:1, :])
                s1sq_row = rowp.tile([1, 512], F32, tag="s1sq")
                nc.vector.tensor_mul(s1sq_row[:], s1_row[:], s1_row[:])
                u_row = rowp.tile([1, 512], F32, tag="u")
                nc.vector.scalar_tensor_tensor(
                    out=u_row[:], in0=s1sq_row[:], scalar=1.0 / HID,
                    in1=stats_ps[32:33, :], op0=OP.mult, op1=OP.subtract,
                )  # u = s1^2/HID - s2  (= -HID*var)
                std_row = rowp.tile([1, 512], F32, tag="std")
                nc.scalar.activation(
                    std_row[:], u_row[:], AF.Sqrt, bias=eps_t[:], scale=-1.0 / HID
                )  # sqrt(var + eps)
                rstd_row = rowp.tile([1, 512], F32, tag="rstd")
                nc.vector.reciprocal_approx_fast(out=rstd_row[:], in_=std_row[:])
                mu_neg = rowp.tile([1, 512], BF16, tag="mu_neg")
                nc.vector.tensor_scalar_mul(
                    out=mu_neg[:], in0=s1_row[:], scalar1=-1.0 / HID
                )
                rstd_b = bcp.tile([128, 512], F32, tag="rstd_b")
                nc.gpsimd.partition_broadcast(rstd_b[:], rstd_row[:])

                if carry is not None:
                    emit_qkv(*carry)
                carry = (xa, rstd_b, mu_neg, ps)
            emit_qkv(*carry)

        # ================= phase 2: attention + pipelined dense ==============
        # keep the PE warm across the pool-release barrier: junk matmuls into
        # a ctx slot (later overwritten by the first start=True accumulate)
        warm_ps = ctps.tile([128, 512], F32, tag="ctx", bufs=2)
        for w in range(12):
            nc.tensor.matmul(
                warm_ps[:], lhsT=ident[:], rhs=qT_sb[:, 512 * (w % 8) : 512 * (w % 8) + 512],
                start=True, stop=True,
            )

        with tc.tile_pool(name="scps", bufs=1, space="PSUM") as scps, \
             tc.tile_pool(name="dsps", bufs=1, space="PSUM") as dsps:

            def emit_dense(qs):
                for c4 in range(4):
                    pc = qs + c4 * 128
                    for half in range(2):
                        ops_ = dsps.tile([128, 512], F32, tag="ds", bufs=2)
                        nc.tensor.matmul(
                            ops_[:], lhsT=ctxT_sb[:, pc : pc + 128],
                            rhs=wd_sb[:, half * 512 : (half + 1) * 512],
                            start=True, stop=True,
                        )
                        osb = obp.tile([128, 512], F16, tag="ob")
                        nc.vector.tensor_copy(osb[:], ops_[:])
                        nc.sync.dma_start(
                            out=out[pc : pc + 128, half * 512 : (half + 1) * 512],
                            in_=osb[:],
                        )

            for b in range(B):
                for qg in range(QG):
                    qs = b * S + qg * 512
                    ctxA_ps = ctps.tile([128, 512], F32, tag="ctx", bufs=2)
                    ctxB_ps = ctps.tile([128, 512], F32, tag="ctx", bufs=2)
                    # per-head sub-units (scores -> exp -> pv) so the two sc
                    # slots recycle alternately and ScalarE never bubbles
                    for kc2 in range(KC // 2):
                        kc = 2 * kc2
                        ks = b * S + kc * 128
                        st = kc == 0
                        sp2 = kc + 1 == KC - 1
                        for h, (kh, vh, cps) in enumerate(
                            ((slice(0, 64), vA_sb, ctxA_ps),
                             (slice(64, 128), vB_sb, ctxB_ps))
                        ):
                            psH = scps.tile(
                                [128, 1024], F32, tag="sc", bufs=2, name=f"ps{h}"
                            )
                            for j in range(2):
                                nc.tensor.matmul(
                                    psH[:, 512 * j : 512 * (j + 1)],
                                    lhsT=kT_sb[kh, ks + 128 * j : ks + 128 * (j + 1)],
                                    rhs=qT_sb[kh, qs : qs + 512],
                                    start=True, stop=True,
                                )
                            eH = etp.tile([128, 1024], BF16, tag="e", name=f"e{h}")
                            if fused_mask:
                                nc.scalar.activation(eH[:], psH[:], AF.Exp)
                            else:
                                for j in range(2):
                                    mcol = madd_sb[
                                        :, b * KC + kc + j : b * KC + kc + j + 1
                                    ]
                                    nc.scalar.activation(
                                        eH[:, 512 * j : 512 * (j + 1)],
                                        psH[:, 512 * j : 512 * (j + 1)],
                                        AF.Exp, bias=mcol, scale=1.0,
                                    )
                            for j in range(2):
                                nc.tensor.matmul(
                                    cps[:, :],
                                    lhsT=vh[:, b * KC + kc + j, :],
                                    rhs=eH[:, 512 * j : 512 * (j + 1)],
                                    start=(st and j == 0), stop=(sp2 and j == 1),
                                )

                    # evacuate ctx PSUM fast (frees the accum slots for the
                    # next query group), then normalize from the SBUF copies:
                    # rdn = exp(-ln(dn)) with dn in row HD of each copy.
                    # head A ctx -> partitions 0:64, head B ctx -> 64:128 so the
                    # normalize muls have partition-aligned SBUF operands
                    cAB = dnp.tile([128, 512], F32, tag="cs", bufs=2)
                    nc.vector.tensor_copy(cAB[0:HD, :], ctxA_ps[0:HD, :])
                    nc.vector.tensor_copy(cAB[HD : 2 * HD, :], ctxB_ps[0:HD, :])
                    dn_row = dnp.tile([1, 1024], F32, tag="dn_row", bufs=2)
                    nc.vector.tensor_copy(dn_row[:, 0:512], ctxA_ps[HD : HD + 1, :])
                    nc.vector.tensor_copy(dn_row[:, 512:1024], ctxB_ps[HD : HD + 1, :])
                    rdn_row = dnp.tile([1, 1024], F32, tag="rdn_row", bufs=1)
                    nc.vector.reciprocal_approx_fast(out=rdn_row[:], in_=dn_row[:])
                    rdn_b = dnp.tile([128, 1024], F32, tag="rdn_b", bufs=1)
                    nc.gpsimd.partition_broadcast(rdn_b[:], rdn_row[:])
                    nc.vector.tensor_mul(
                        ctxT_sb[0:HD, qs : qs + 512],
                        cAB[0:HD, :], rdn_b[0:HD, 0:512],
                    )
                    nc.vector.tensor_mul(
                        ctxT_sb[HD : 2 * HD, qs : qs + 512],
                        cAB[HD : 2 * HD, :], rdn_b[HD : 2 * HD, 512:1024],
                    )
                    emit_dense(qs)
    nc.compile()
    return nc


def _get_nc(with_bias, fused_mask):
    key = (bool(with_bias), bool(fused_mask))
    if key not in _BUILT:
        _BUILT[key] = _build(*key)
    return _BUILT[key]


def kernel(
    hidden_states,
    attention_mask,
    Wq, bq, Wk, bk, Wv, bv, Wd, bd,
    ln_gamma, ln_beta,
):
    from concourse.bass_utils import run_bass_kernel_spmd

    hidden_states = np.asarray(hidden_states, dtype=np.float32)
    attention_mask = np.asarray(attention_mask, dtype=np.float32)
    Wq, bq = np.asarray(Wq, np.float32), np.asarray(bq, np.float32)
    Wk, bk = np.asarray(Wk, np.float32), np.asarray(bk, np.float32)
    Wv, bv = np.asarray(Wv, np.float32), np.asarray(bv, np.float32)
    Wd, bd = np.asarray(Wd, np.float32), np.asarray(bd, np.float32)
    gamma = np.asarray(ln_gamma, np.float32)
    beta = np.asarray(ln_beta, np.float32)

    x2d = hidden_states.reshape(PB, HID)
    xT = np.ascontiguousarray(x2d.T).astype(ml_dtypes.bfloat16)

    ma = (-1000.0 * (1.0 - attention_mask)).astype(np.float32)  # [B, S]
    madd = np.ascontiguousarray(
        ma.reshape(B, KC, 128).transpose(2, 0, 1).reshape(128, B * KC)
    )
    fused_mask = not np.any(ma != 0)

    in_maps = []
    biases_eff = []
    for p in range(NCORES):
        sl = slice(DL * p, DL * (p + 1))
        wq_e = Wq[sl, :] * gamma[None, :] * np.float32(SCALE)
        wk_e = Wk[sl, :] * gamma[None, :]
        wv_e = Wv[sl, :] * gamma[None, :]
        wq_b = np.ascontiguousarray(wq_e.T).astype(ml_dtypes.bfloat16)
        wk_b = np.ascontiguousarray(wk_e.T).astype(ml_dtypes.bfloat16)
        wv_b = np.ascontiguousarray(wv_e.T).astype(ml_dtypes.bfloat16)
        # raw column sums of the bf16 weights actually used on device,
        # as three [1, DL] rows for the rank-1 mean-correction matmul
        wsumsT = np.concatenate(
            [
                wq_b.astype(np.float32).sum(axis=0),
                wk_b.astype(np.float32).sum(axis=0),
                wv_b.astype(np.float32).sum(axis=0),
            ]
        ).reshape(1, 3 * DL).astype(ml_dtypes.bfloat16)
        b_eff = np.stack(
            [
                (Wq[sl, :] @ beta + bq[sl]) * np.float32(SCALE),
                Wk[sl, :] @ beta + bk[sl],
                Wv[sl, :] @ beta + bv[sl],
            ],
            axis=1,
        ).astype(np.float32)
        biases_eff.append(b_eff)
        wd_s = np.ascontiguousarray(Wd[:, sl].T).astype(ml_dtypes.bfloat16)
        in_maps.append(
            {
                "xT": xT,
                "wq": wq_b,
                "wk": wk_b,
                "wv": wv_b,
                "wd": wd_s,
                "wsumsT": wsumsT,
            }
        )

    with_bias = any(np.any(b != 0) for b in biases_eff)
    if with_bias:
        for p in range(NCORES):
            in_maps[p]["bqkv"] = biases_eff[p]
    if not fused_mask:
        for p in range(NCORES):
            in_maps[p]["madd"] = madd

    nc = _get_nc(with_bias, fused_mask)
    last_launch["nc"] = nc
    last_launch["in_maps"] = in_maps
    res = run_bass_kernel_spmd(nc, in_maps, core_ids=list(range(NCORES)))
    acc = res.results[0]["out"].astype(np.float32).copy()
    for p in range(1, NCORES):
        acc += res.results[p]["out"]
    acc += bd[None, :]
    return acc.reshape(B, S, HID)


# revision 30
# speedup vs baseline: 1.3878x; 1.0202x over previous
"""Fused pre-LN multi-head self-attention block for Trainium2, SPMD over 8 NeuronCores.

Strategy (Megatron-style head parallelism):
  - Each core owns 2 of the 16 heads (a 128-wide slice of the QKV output dims)
    for BOTH batches, and computes a partial dense-projection output; the host
    sums the 8 partials and adds bd.
  - hidden_states is shipped transposed (xT [1024, 4096], bf16) so LayerNorm
    statistics are computed with ones-matmuls on the PE and the normalization
    itself is algebraically folded into the QKV projections:
        q = rstd * (x @ WqeT - mu * wqsum) + bqe
    with Wqe = Wq_slice * gamma * scale (host-folded), bqe = Wq_slice @ beta + bq.
  - Scores are computed transposed (sT[k,q] = kT.T @ qT); softmax skips the
    max-subtraction (inputs are standard-normal scale); the denominator comes
    from a ones-column inside the padded V tiles in the probs@V matmul.
  - ScalarE runs exactly one activation function per phase (Sqrt in phase 1,
    Exp in phase 2) so each phase loads its ACT table set once; both
    reciprocals (1/std, 1/denominator) use the single-op DVE
    reciprocal_approx_fast (~18-bit accurate, ~5x faster than the iterative
    DVE reciprocal that dominated the old kernel's vector-engine time).
  - x^2 for the variance stats is split between VectorE and GpSimd to
    balance engine load in phase 1.
  - PSUM pools are scoped per phase: phase 1 (stats 2 + qkv 3 + vT 2 banks)
    is released before phase 2 allocates (scores 4 + ctx 2 + dense 2 banks),
    so each phase gets the banks it needs without static over-commit.
  - V tiles are zero-padded to 128 lhsT columns (ones col at 64) so the
    probs@V weight loads qualify for fast-weight-load and overlap matmuls.
"""

import sys

sys.path.insert(0, "/opt/trn_rl_repo")

import numpy as np
import ml_dtypes

B, S, HID = 2, 2048, 1024
NH, HD = 16, 64
EPS = 1e-12
NCORES = 8
DL = HID // NCORES          # 128 local q/k/v dims (2 heads) per core
PB = B * S                  # 4096 total positions
SCALE = 1.0 / np.sqrt(HD)   # combined q*k score scale (1/8), folded into Wq
NPOSG = PB // 512           # 8 position groups of 512
KC = S // 128               # 16 key chunks per batch
QG = S // 512               # 4 query groups per batch

_BUILT = {}
last_launch = {}


def _build(with_bias, fused_mask):
    import concourse.tile as tile
    from concourse import bacc, mybir
    from contextlib import ExitStack

    F32 = mybir.dt.float32
    BF16 = mybir.dt.bfloat16
    AF = mybir.ActivationFunctionType
    OP = mybir.AluOpType

    nc = bacc.Bacc("TRN2", target_bir_lowering=False, debug=False)

    xT = nc.dram_tensor("xT", [HID, PB], BF16, kind="ExternalInput")
    wq = nc.dram_tensor("wq", [HID, DL], BF16, kind="ExternalInput")
    wk = nc.dram_tensor("wk", [HID, DL], BF16, kind="ExternalInput")
    wv = nc.dram_tensor("wv", [HID, DL], BF16, kind="ExternalInput")
    wd = nc.dram_tensor("wd", [DL, HID], BF16, kind="ExternalInput")
    # raw bf16-weight column sums as three [1, DL] rows (q, k, v) for the
    # rank-1 mean-correction matmul
    wsumsT = nc.dram_tensor("wsumsT", [1, 3 * DL], BF16, kind="ExternalInput")
    if not fused_mask:
        madd = nc.dram_tensor("madd", [128, B * KC], F32, kind="ExternalInput")
    if with_bias:
        bqkv = nc.dram_tensor("bqkv", [DL, 3], F32, kind="ExternalInput")
    F16 = mybir.dt.float16
    out = nc.dram_tensor("out", [PB, HID], F16, kind="ExternalOutput")

    with tile.TileContext(nc) as tc, ExitStack() as ctx:
        consts = ctx.enter_context(tc.tile_pool(name="consts", bufs=1))
        persist = ctx.enter_context(tc.tile_pool(name="persist", bufs=1))
        xpool = ctx.enter_context(tc.tile_pool(name="xpool", bufs=2))
        sqp = ctx.enter_context(tc.tile_pool(name="sqp", bufs=2))
        rowp = ctx.enter_context(tc.tile_pool(name="rowp", bufs=8))
        bcp = ctx.enter_context(tc.tile_pool(name="bcp", bufs=2))
        epp = ctx.enter_context(tc.tile_pool(name="epp", bufs=3))
        etp = ctx.enter_context(tc.tile_pool(name="etp", bufs=4))
        dnp = ctx.enter_context(tc.tile_pool(name="dnp", bufs=2))
        obp = ctx.enter_context(tc.tile_pool(name="obp", bufs=3))

        # ---- constants / weights
        ones_col = consts.tile([128, 1], BF16)
        nc.vector.memset(ones_col[:], 1.0)
        ident = consts.tile([128, 128], BF16)
        from concourse.masks import make_identity
        make_identity(nc, ident[:])
        eps_t = consts.tile([1, 1], F32)
        nc.vector.memset(eps_t[:], EPS)
        wsumsT_sb = consts.tile([1, 3 * DL], BF16)
        nc.sync.dma_start(out=wsumsT_sb[:], in_=wsumsT[:, :])
        if not fused_mask:
            madd_sb = consts.tile([128, B * KC], F32)
            nc.sync.dma_start(out=madd_sb[:], in_=madd[:, :])
        if with_bias:
            bqkv_sb = consts.tile([DL, 3], F32)
            nc.sync.dma_start(out=bqkv_sb[:], in_=bqkv[:, :])

        xT_r = xT.rearrange("(hc p) q -> p hc q", p=128)
        x_first = xpool.tile([128, 8, 512], BF16, tag="x")
        nc.sync.dma_start(out=x_first[:], in_=xT_r[:, :, 0:512])

        wq_sb = persist.tile([128, 8, DL], BF16)
        wk_sb = persist.tile([128, 8, DL], BF16)
        wv_sb = persist.tile([128, 8, DL], BF16)
        for wsb, wdr in ((wq_sb, wq), (wk_sb, wk), (wv_sb, wv)):
            nc.sync.dma_start(
                out=wsb[:], in_=wdr.rearrange("(hc p) d -> p hc d", p=128)
            )
        wd_sb = persist.tile([DL, HID], BF16)
        nc.sync.dma_start(out=wd_sb[:], in_=wd[:, :])

        qT_sb = persist.tile([128, PB], BF16)
        kT_sb = persist.tile([128, PB], BF16)
        # v tiles padded to 128 lhsT columns: [0:64]=v, [64]=ones, [65:128]=0
        vA_sb = persist.tile([128, B * KC, 128], BF16)
        vB_sb = persist.tile([128, B * KC, 128], BF16)
        for vsb in (vA_sb, vB_sb):
            nc.vector.memset(vsb[:, :, HD : HD + 1], 1.0)
            nc.vector.memset(vsb[:, :, HD + 1 : 128], 0.0)
        ctxT_sb = persist.tile([128, PB], BF16)

        # ================= phase 1: LN stats + QKV projections ===============
        # software-pipelined: stats(pg) runs on the PE before qkv(pg-1), so
        # the rank-1 mean-correction matmul of pg-1 never waits on the DVE
        # row chain (in-order PE never stalls).
        with tc.tile_pool(name="ph1ps", bufs=1, space="PSUM") as ph1:
            x_prev = x_first
            carry = None  # (xa, rstd_b, mu_neg, ps) of the previous pg

            def emit_qkv(xa, rstd_b, mu_neg, ps):
                vT_blk = epp.tile([128, 512], BF16, tag="vT")
                for (w_sb, wcol, target) in (
                    (wq_sb, 0, qT_sb[:, ps : ps + 512]),
                    (wk_sb, 1, kT_sb[:, ps : ps + 512]),
                    (wv_sb, 2, vT_blk[:]),
                ):
                    mm_ps = ph1.tile([128, 512], F32, tag="mm", bufs=3)
                    for hc in range(8):
                        nc.tensor.matmul(
                            mm_ps[:], lhsT=w_sb[:, hc, :], rhs=xa[:, hc, :],
                            start=(hc == 0), stop=False,
                        )
                    nc.tensor.matmul(
                        mm_ps[:],
                        lhsT=wsumsT_sb[:, wcol * DL : (wcol + 1) * DL],
                        rhs=mu_neg[:], start=False, stop=True,
                    )
                    # target = raw_c * rstd
                    if with_bias:
                        t2 = epp.tile([128, 512], F32, tag="ep2")
                        nc.vector.scalar_tensor_tensor(
                            out=t2[:], in0=rstd_b[:], scalar=1.0,
                            in1=mm_ps[:], op0=OP.mult, op1=OP.mult,
                        )
                        nc.vector.tensor_scalar_add(
                            out=target, in0=t2[:], scalar1=bqkv_sb[:, wcol : wcol + 1]
                        )
                    else:
                        nc.vector.scalar_tensor_tensor(
                            out=target, in0=rstd_b[:], scalar=1.0,
                            in1=mm_ps[:], op0=OP.mult, op1=OP.mult,
                        )
                # transpose vT -> v (per 128-pos chunk) into padded v tiles
                for c4 in range(4):
                    j = (ps // 512) * 4 + c4  # global 128-chunk == b*KC + kc
                    tp_ps = ph1.tile([128, 128], BF16, tag="vt", bufs=2)
                    nc.tensor.transpose(
                        tp_ps[:, :], vT_blk[:, c4 * 128 : (c4 + 1) * 128], ident[:]
                    )
                    nc.vector.tensor_copy(vA_sb[:, j, 0:HD], tp_ps[:, 0:HD])
                    nc.vector.tensor_copy(vB_sb[:, j, 0:HD], tp_ps[:, HD : 2 * HD])

            for pg in range(NPOSG):
                ps = pg * 512
                xa = x_prev
                if pg + 1 < NPOSG:
                    x_nxt = xpool.tile([128, 8, 512], BF16, tag="x")
                    nc.sync.dma_start(
                        out=x_nxt[:], in_=xT_r[:, :, ps + 512 : ps + 1024]
                    )
                    x_prev = x_nxt

                # --- stats chain (sum in psum row 0, sumsq in row 32);
                # x^2 in two wide DVE muls (cheaper than 8 narrow ones)
                stats_ps = ph1.tile([64, 512], F32, tag="stats", bufs=2)
                xsq_h = [
                    sqp.tile([128, 4, 512], BF16, tag="xsq", name=f"xsq{h}")
                    for h in range(2)
                ]
                for h in range(2):
                    nc.vector.tensor_mul(
                        xsq_h[h][:], xa[:, 4 * h : 4 * h + 4, :],
                        xa[:, 4 * h : 4 * h + 4, :],
                    )
                for hc in range(8):
                    st = hc == 0
                    sp = hc == 7
                    nc.tensor.matmul(
                        stats_ps[0:1, :], lhsT=ones_col[:], rhs=xa[:, hc, :],
                        start=st, stop=sp, skip_group_check=True,
                    )
                    nc.tensor.matmul(
                        stats_ps[32:33, :], lhsT=ones_col[:],
                        rhs=xsq_h[hc // 4][:, hc % 4, :],
                        start=st, stop=sp, skip_group_check=True,
                    )

                # row stats: s1 (sum); rstd = 1/sqrt(var+eps)
                s1_row = rowp.tile([1, 512], F32, tag="s1")
                nc.vector.tensor_copy(s1_row[:], stats_ps[0:1, :])
                s1sq_row = rowp.tile([1, 512], F32, tag="s1sq")
                nc.vector.tensor_mul(s1sq_row[:], s1_row[:], s1_row[:])
                u_row = rowp.tile([1, 512], F32, tag="u")
                nc.vector.scalar_tensor_tensor(
                    out=u_row[:], in0=s1sq_row[:], scalar=1.0 / HID,
                    in1=stats_ps[32:33, :], op0=OP.mult, op1=OP.subtract,
                )  # u = s1^2/HID - s2  (= -HID*var)
                std_row = rowp.tile([1, 512], F32, tag="std")
                nc.scalar.activation(
                    std_row[:], u_row[:], AF.Sqrt, bias=eps_t[:], scale=-1.0 / HID
                )  # sqrt(var + eps)
                rstd_row = rowp.tile([1, 512], F32, tag="rstd")
                nc.vector.reciprocal_approx_fast(out=rstd_row[:], in_=std_row[:])
                mu_neg = rowp.tile([1, 512], BF16, tag="mu_neg")
                nc.vector.tensor_scalar_mul(
                    out=mu_neg[:], in0=s1_row[:], scalar1=-1.0 / HID
                )
                rstd_b = bcp.tile([128, 512], F32, tag="rstd_b")
                nc.gpsimd.partition_broadcast(rstd_b[:], rstd_row[:])

                if carry is not None:
                    emit_qkv(*carry)
                carry = (xa, rstd_b, mu_neg, ps)
            emit_qkv(*carry)

        # ================= phase 2: attention + pipelined dense ==============
        with tc.tile_pool(name="scps", bufs=1, space="PSUM") as scps, \
             tc.tile_pool(name="ctps", bufs=1, space="PSUM") as ctps, \
             tc.tile_pool(name="dsps", bufs=1, space="PSUM") as dsps:

            def emit_dense(qs):
                for c4 in range(4):
                    pc = qs + c4 * 128
                    for half in range(2):
                        ops_ = dsps.tile([128, 512], F32, tag="ds", bufs=2)
                        nc.tensor.matmul(
                            ops_[:], lhsT=ctxT_sb[:, pc : pc + 128],
                            rhs=wd_sb[:, half * 512 : (half + 1) * 512],
                            start=True, stop=True,
                        )
                        osb = obp.tile([128, 512], F16, tag="ob")
                        nc.vector.tensor_copy(osb[:], ops_[:])
                        nc.sync.dma_start(
                            out=out[pc : pc + 128, half * 512 : (half + 1) * 512],
                            in_=osb[:],
                        )

            for b in range(B):
                for qg in range(QG):
                    qs = b * S + qg * 512
                    ctxA_ps = ctps.tile([128, 512], F32, tag="ctx", bufs=2)
                    ctxB_ps = ctps.tile([128, 512], F32, tag="ctx", bufs=2)
                    # per-head sub-units (scores -> exp -> pv) so the two sc
                    # slots recycle alternately and ScalarE never bubbles
                    for kc2 in range(KC // 2):
                        kc = 2 * kc2
                        ks = b * S + kc * 128
                        st = kc == 0
                        sp2 = kc + 1 == KC - 1
                        for h, (kh, vh, cps) in enumerate(
                            ((slice(0, 64), vA_sb, ctxA_ps),
                             (slice(64, 128), vB_sb, ctxB_ps))
                        ):
                            psH = scps.tile(
                                [128, 1024], F32, tag="sc", bufs=2, name=f"ps{h}"
                            )
                            for j in range(2):
                                nc.tensor.matmul(
                                    psH[:, 512 * j : 512 * (j + 1)],
                                    lhsT=kT_sb[kh, ks + 128 * j : ks + 128 * (j + 1)],
                                    rhs=qT_sb[kh, qs : qs + 512],
                                    start=True, stop=True,
                                )
                            eH = etp.tile([128, 1024], BF16, tag="e", name=f"e{h}")
                            if fused_mask:
                                nc.scalar.activation(eH[:], psH[:], AF.Exp)
                            else:
                                for j in range(2):
                                    mcol = madd_sb[
                                        :, b * KC + kc + j : b * KC + kc + j + 1
                                    ]
                                    nc.scalar.activation(
                                        eH[:, 512 * j : 512 * (j + 1)],
                                        psH[:, 512 * j : 512 * (j + 1)],
                                        AF.Exp, bias=mcol, scale=1.0,
                                    )
                            for j in range(2):
                                nc.tensor.matmul(
                                    cps[:, :],
                                    lhsT=vh[:, b * KC + kc + j, :],
                                    rhs=eH[:, 512 * j : 512 * (j + 1)],
                                    start=(st and j == 0), stop=(sp2 and j == 1),
                                )

                    # evacuate ctx PSUM fast (frees the accum slots for the
                    # next query group), then normalize from the SBUF copies:
                    # rdn = exp(-ln(dn)) with dn in row HD of each copy.
                    # head A ctx -> partitions 0:64, head B ctx -> 64:128 so the
                    # normalize muls have partition-aligned SBUF operands
                    cAB = dnp.tile([128, 512], F32, tag="cs", bufs=2)
                    nc.vector.tensor_copy(cAB[0:HD, :], ctxA_ps[0:HD, :])
                    nc.vector.tensor_copy(cAB[HD : 2 * HD, :], ctxB_ps[0:HD, :])
                    dn_row = dnp.tile([1, 1024], F32, tag="dn_row", bufs=2)
                    nc.vector.tensor_copy(dn_row[:, 0:512], ctxA_ps[HD : HD + 1, :])
                    nc.vector.tensor_copy(dn_row[:, 512:1024], ctxB_ps[HD : HD + 1, :])
                    rdn_row = dnp.tile([1, 1024], F32, tag="rdn_row", bufs=1)
                    nc.vector.reciprocal_approx_fast(out=rdn_row[:], in_=dn_row[:])
                    rdn_b = dnp.tile([128, 1024], F32, tag="rdn_b", bufs=1)
                    nc.gpsimd.partition_broadcast(rdn_b[:], rdn_row[:])
                    nc.vector.tensor_mul(
                        ctxT_sb[0:HD, qs : qs + 512],
                        cAB[0:HD, :], rdn_b[0:HD, 0:512],
                    )
                    nc.vector.tensor_mul(
                        ctxT_sb[HD : 2 * HD, qs : qs + 512],
                        cAB[HD : 2 * HD, :], rdn_b[HD : 2 * HD, 512:1024],
                    )
                    emit_dense(qs)
    nc.compile()
    return nc


def _get_nc(with_bias, fused_mask):
    key = (bool(with_bias), bool(fused_mask))
    if key not in _BUILT:
        _BUILT[key] = _build(*key)
    return _BUILT[key]


def kernel(
    hidden_states,
    attention_mask,
    Wq, bq, Wk, bk, Wv, bv, Wd, bd,
    ln_gamma, ln_beta,
):
    from concourse.bass_utils import run_bass_kernel_spmd

    hidden_states = np.asarray(hidden_states, dtype=np.float32)
    attention_mask = np.asarray(attention_mask, dtype=np.float32)
    Wq, bq = np.asarray(Wq, np.float32), np.asarray(bq, np.float32)
    Wk, bk = np.asarray(Wk, np.float32), np.asarray(bk, np.float32)
    Wv, bv = np.asarray(Wv, np.float32), np.asarray(bv, np.float32)
    Wd, bd = np.asarray(Wd, np.float32), np.asarray(bd, np.float32)
    gamma = np.asarray(ln_gamma, np.float32)
    beta = np.asarray(ln_beta, np.float32)

    x2d = hidden_states.reshape(PB, HID)
    xT = np.ascontiguousarray(x2d.T).astype(ml_dtypes.bfloat16)

    ma = (-1000.0 * (1.0 - attention_mask)).astype(np.float32)  # [B, S]
    madd = np.ascontiguousarray(
        ma.reshape(B, KC, 128).transpose(2, 0, 1).reshape(128, B * KC)
    )
    fused_mask = not np.any(ma != 0)

    in_maps = []
    biases_eff = []
    for p in range(NCORES):
        sl = slice(DL * p, DL * (p + 1))
        wq_e = Wq[sl, :] * gamma[None, :] * np.float32(SCALE)
        wk_e = Wk[sl, :] * gamma[None, :]
        wv_e = Wv[sl, :] * gamma[None, :]
        wq_b = np.ascontiguousarray(wq_e.T).astype(ml_dtypes.bfloat16)
        wk_b = np.ascontiguousarray(wk_e.T).astype(ml_dtypes.bfloat16)
        wv_b = np.ascontiguousarray(wv_e.T).astype(ml_dtypes.bfloat16)
        # raw column sums of the bf16 weights actually used on device,
        # as three [1, DL] rows for the rank-1 mean-correction matmul
        wsumsT = np.concatenate(
            [
                wq_b.astype(np.float32).sum(axis=0),
                wk_b.astype(np.float32).sum(axis=0),
                wv_b.astype(np.float32).sum(axis=0),
            ]
        ).reshape(1, 3 * DL).astype(ml_dtypes.bfloat16)
        b_eff = np.stack(
            [
                (Wq[sl, :] @ beta + bq[sl]) * np.float32(SCALE),
                Wk[sl, :] @ beta + bk[sl],
                Wv[sl, :] @ beta + bv[sl],
            ],
            axis=1,
        ).astype(np.float32)
        biases_eff.append(b_eff)
        wd_s = np.ascontiguousarray(Wd[:, sl].T).astype(ml_dtypes.bfloat16)
        in_maps.append(
            {
                "xT": xT,
                "wq": wq_b,
                "wk": wk_b,
                "wv": wv_b,
                "wd": wd_s,
                "wsumsT": wsumsT,
            }
        )

    with_bias = any(np.any(b != 0) for b in biases_eff)
    if with_bias:
        for p in range(NCORES):
            in_maps[p]["bqkv"] = biases_eff[p]
    if not fused_mask:
        for p in range(NCORES):
            in_maps[p]["madd"] = madd

    nc = _get_nc(with_bias, fused_mask)
    last_launch["nc"] = nc
    last_launch["in_maps"] = in_maps
    res = run_bass_kernel_spmd(nc, in_maps, core_ids=list(range(NCORES)))
    acc = res.results[0]["out"].astype(np.float32).copy()
    for p in range(1, NCORES):
        acc += res.results[p]["out"]
    acc += bd[None, :]
    return acc.reshape(B, S, HID)
